# revision 1
# baseline (speedup 1.0000x reference)
"""Multi-head causal attention (B=2, S=2048, C=2048, H=16) on 8 NeuronCores.

Sharding: 2-way data parallel over batch x 4-way tensor parallel over heads.
Core i handles batch b = i // 4 and heads [4*(i%4), 4*(i%4)+4).

Design (s-major scores, no denominator matmul):
  phase A: Q/K (d on partitions) and V (s on partitions) projections in f16,
           weights SBUF-cached via bulk sync-queue DMAs, c-inner PSUM
           accumulation (N=512 streams), emitted in 8-matmul halves for
           fine-grained interleaving. Q/K evacuate on ACT (Identity + bias
           AP, SCALE folded into Q), V on DVE. The V bias is folded into
           the host-side reduction (softmax rows sum to 1, so bv only
           contributes Wo @ bv to the output).
  phase B: scores s-major (stationary Q chunk [d,128], moving K) into
           [128,1024] two-bank PSUM pairs; causal mask via DVE add of a
           -100 tile on the diagonal 128-block; exp on ACT with accum_out
           (softmax denominator for free, one op per 1024 cols); the
           denominator reciprocal is appended as an extra f16 column and
           rides the xbar DMA transpose ([s,t] -> [t,s] blocks) that feeds
           the AV matmul (stationary V, N=512 streams); the reciprocal row
           is partition-broadcast (gpsimd) and multiplied in during PSUM
           evacuation (DVE).
  phase C: output projection partials; evacuation alternates ACT/DVE;
           merged f16 stores (one DMA per 512x512 block); bo + Wo @ bv are
           added on the host during the cross-core partial reduction.
  Schedule: B/C units of block sb-1 interleave 1:1 into the 24 phase-A
           half-groups of block sb; unit order keeps transpose(h+2) after
           AV(h) so the 2-deep pet rotation never serializes; sb=3 units
           interleave with phase A of sb=3 itself (hand-ordered tail).
  Queues: sync = bulk loads + transposes + stores (transposes emitted one
           unit late so they reach the queue head with satisfied waits);
           gpsimd = small loads + broadcasts; ACT/DVE split the evacuation
           work to keep either in-order queue from convoying.
"""

import numpy as np

B, S, C, H = 2, 2048, 2048, 16
D = C // H            # 128 per-head dim
HL = 4                # heads per core
ML = HL * D           # 512 local channels
P = 128
NCT = C // P          # 16 contraction tiles
NT = S // P           # 16 key tiles
SCALE = 1.0 / float(np.sqrt(D))

_CACHE = {}


def _build():
    import concourse.bacc as bacc
    import concourse.mybir as mybir
    import concourse.tile as tile

    f32 = mybir.dt.float32
    f16 = mybir.dt.float16
    Exp = mybir.ActivationFunctionType.Exp
    Ident = mybir.ActivationFunctionType.Identity
    Copy = mybir.ActivationFunctionType.Copy
    add = mybir.AluOpType.add
    X = mybir.AxisListType.X

    nc = bacc.Bacc("TRN2", target_bir_lowering=False, debug=False, num_devices=8)

    xt = nc.dram_tensor("xt", [C, S], f16, kind="ExternalInput")       # x[b].T
    wqt = nc.dram_tensor("wqt", [C, ML], f16, kind="ExternalInput")    # Wq.T cols
    wkt = nc.dram_tensor("wkt", [C, ML], f16, kind="ExternalInput")
    wvt = nc.dram_tensor("wvt", [C, ML], f16, kind="ExternalInput")
    wot = nc.dram_tensor("wot", [ML, C], f16, kind="ExternalInput")    # Wo.T rows
    bqs = nc.dram_tensor("bqs", [ML], f32, kind="ExternalInput")       # bq * SCALE
    bk = nc.dram_tensor("bk", [ML], f32, kind="ExternalInput")
    maskd = nc.dram_tensor("maskd", [P, P], f32, kind="ExternalInput") # 0 / -100
    out = nc.dram_tensor("out", [S, C], f16, kind="ExternalOutput")

    xt_r = xt[:, :].rearrange("(c p) s -> p c s", p=P)    # [128, 16, 2048]
    wq_r = wqt[:, :].rearrange("(c p) m -> p c m", p=P)   # [128, 16, 512]
    wk_r = wkt[:, :].rearrange("(c p) m -> p c m", p=P)
    wv_r = wvt[:, :].rearrange("(c p) m -> p c m", p=P)
    wo_r = wot[:, :].rearrange("(m p) j -> p m j", p=P)   # [128, 4, 2048]
    out_r = out[:, :].rearrange("(g st p) j -> g p st j", p=P, st=4)  # [4, 128, 4, 2048]

    with tile.TileContext(nc) as tc:
        with tc.tile_pool(name="persist", bufs=1) as pp_, \
             tc.tile_pool(name="work", bufs=1) as wk, \
             tc.tile_pool(name="psp", bufs=1, space="PSUM") as psp:

            wvc = pp_.tile([P, NCT, ML], f16, tag="wvc", name="wvc")
            wqc = pp_.tile([P, NCT, ML], f16, tag="wqc", name="wqc")
            wkc = pp_.tile([P, NCT, ML], f16, tag="wkc", name="wkc")
            woc = pp_.tile([P, HL, C], f16, tag="woc", name="woc")
            K = [pp_.tile([P, S], f16, tag=f"k{m}", name=f"k{m}") for m in range(HL)]
            V = [pp_.tile([P, ML], f16, tag=f"v{t}", name=f"v{t}") for t in range(NT)]
            maskT = pp_.tile([P, P], f32, tag="maskT", name="maskT")
            bqs_t = [pp_.tile([P, 1], f32, tag=f"bq{m}", name=f"bq{m}") for m in range(HL)]
            bk_t = [pp_.tile([P, 1], f32, tag=f"bk{m}", name=f"bk{m}") for m in range(HL)]

            # small loads on gpsimd; bulk loads on the fast sync HWDGE path
            nc.gpsimd.dma_start(maskT[:], maskd[:, :])
            for m in range(HL):
                nc.gpsimd.dma_start(bqs_t[m][:], bqs[m * P:(m + 1) * P, None])
                nc.gpsimd.dma_start(bk_t[m][:], bk[m * P:(m + 1) * P, None])

            # xt cache: contraction-split halves (c 0..7 / 8..15) per s-block
            xtc_tiles = {}

            def load_xtc(sb, ch, quarters=False):
                tag = "xtcA" if ch == 0 else "xtcB"
                bufs = 2 if ch == 0 else 1
                t = wk.tile([P, 8, 512], f16, tag=tag, bufs=bufs,
                            name=f"xtc{sb}{ch}")
                xtc_tiles[(sb, ch)] = t
                s0 = sb * 512
                if quarters:
                    for q in range(2):
                        nc.sync.dma_start(
                            t[:, q * 4:(q + 1) * 4, :],
                            xt_r[:, ch * 8 + q * 4:ch * 8 + (q + 1) * 4,
                                 s0:s0 + 512])
                else:
                    nc.sync.dma_start(t[:], xt_r[:, ch * 8:(ch + 1) * 8,
                                                  s0:s0 + 512])
                return t

            # startup: wv/xt loads interleaved in first-use order
            nc.sync.dma_start(wvc[:, 0:4, :], wv_r[:, 0:4, :])
            t0 = wk.tile([P, 8, 512], f16, tag="xtcA", bufs=2, name="xtc00")
            xtc_tiles[(0, 0)] = t0
            nc.sync.dma_start(t0[:, 0:4, :], xt_r[:, 0:4, 0:512])
            nc.sync.dma_start(wvc[:, 4:8, :], wv_r[:, 4:8, :])
            nc.sync.dma_start(t0[:, 4:8, :], xt_r[:, 4:8, 0:512])
            nc.sync.dma_start(wvc[:, 8:12, :], wv_r[:, 8:12, :])
            load_xtc(0, 1)
            nc.sync.dma_start(wvc[:, 12:16, :], wv_r[:, 12:16, :])
            nc.sync.dma_start(wqc[:], wq_r)
            nc.sync.dma_start(wkc[:], wk_r)
            nc.sync.dma_start(woc[:], wo_r)

            state = {}

            # ---------------- phase A groups (emitted in 8-MM halves) ----
            def a_group(sb, kind, idx, half, last_group=False):
                def go():
                    xa = xtc_tiles[(sb, 0)]
                    xb = xtc_tiles[(sb, 1)]
                    order = list(range(16))
                    if last_group:
                        order = list(range(8, 16)) + list(range(8))
                    if half == 0:
                        acc = psp.tile([P, 512], f32, tag="pa", bufs=2,
                                       name=f"pa_{sb}{kind}{idx}")
                        state[("pa", sb, kind, idx)] = acc
                    else:
                        acc = state[("pa", sb, kind, idx)]
                    for n in range(half * 8, half * 8 + 8):
                        c = order[n]
                        ch, c8 = divmod(c, 8)
                        xtile = xa if ch == 0 else xb
                        if kind == "v":
                            nc.tensor.matmul(acc[:],
                                             xtile[:, c8, idx * P:(idx + 1) * P],
                                             wvc[:, c, :],
                                             start=(n == 0), stop=(n == 15))
                        else:
                            wc = wqc if kind == "q" else wkc
                            nc.tensor.matmul(acc[:],
                                             wc[:, c, idx * P:(idx + 1) * P],
                                             xtile[:, c8, :],
                                             start=(n == 0), stop=(n == 15))
                    if half == 0:
                        return
                    if kind == "v":
                        nc.vector.tensor_copy(V[sb * 4 + idx][:], acc[:])
                    elif kind == "q":
                        qsb = state[("q", sb)]
                        nc.scalar.activation(qsb[idx][:], acc[:], Ident,
                                             bias=bqs_t[idx][:], scale=SCALE)
                    else:
                        s0 = sb * 512
                        nc.scalar.activation(K[idx][:, s0:s0 + 512], acc[:],
                                             Ident, bias=bk_t[idx][:])
                return go

            # ---------------- phase B/C units ----------------
            def b_scores(sb, h, scs=(0, 1, 2, 3)):
                """scores + exp + denominator for head h (no transpose)."""
                def go():
                    s0 = sb * 512
                    qsb = state[("q", sb)]
                    for sc in scs:
                        t_end = s0 + sc * P + P
                        ntt = (t_end + 511) // 512
                        pes = wk.tile([P, (14 + sc) * P], f16, tag=f"pes{sc}",
                                      bufs=2, name=f"pes{sb}{h}{sc}")
                        den4 = wk.tile([P, 2], f32, tag=f"den{sc}", bufs=2,
                                       name=f"den{sb}{h}{sc}")
                        rec1 = wk.tile([P, 1], f32, tag=f"rec{sc}", bufs=2,
                                       name=f"rec{sb}{h}{sc}")
                        ps = None
                        for tt in range(ntt):
                            w = min(512, t_end - tt * 512)
                            tp, off = divmod(tt, 2)
                            if off == 0:
                                ps = psp.tile([P, 1024], f32, tag="ps", bufs=2,
                                              name=f"ps{sb}{h}{sc}{tp}")
                            nc.tensor.matmul(ps[:, off * 512:off * 512 + w],
                                             qsb[h][:, sc * P:(sc + 1) * P],
                                             K[h][:, tt * 512:tt * 512 + w],
                                             start=True, stop=True)
                            if tt == ntt - 1:
                                w2 = off * 512 + w
                                nc.vector.tensor_add(ps[:, w2 - P:w2],
                                                     ps[:, w2 - P:w2], maskT[:])
                            if off == 1 or tt == ntt - 1:
                                w2 = off * 512 + w
                                nc.scalar.activation(
                                    pes[:, tp * 1024:tp * 1024 + w2],
                                    ps[:, :w2], Exp,
                                    accum_out=den4[:, tp:tp + 1])
                        ntp = (ntt + 1) // 2
                        if ntp > 1:
                            dsum = wk.tile([P, 1], f32, tag=f"dsum{sc}", bufs=2,
                                           name=f"dsum{sb}{h}{sc}")
                            nc.vector.tensor_reduce(dsum[:], den4[:, :ntp], X, add)
                            nc.vector.reciprocal(rec1[:], dsum[:])
                        else:
                            nc.vector.reciprocal(rec1[:], den4[:, 0:1])
                        nc.vector.tensor_copy(pes[:, t_end:t_end + 1], rec1[:])
                        state[("pes", sb, h, sc)] = pes
                return go

            def b_transpose(sb, h, scs=(0, 1, 2, 3)):
                """xbar transposes for head h (emitted one unit late)."""
                def go():
                    s0 = sb * 512
                    if 0 in scs:
                        pet = wk.tile([P, 17, 512], f16, tag="pet", bufs=2,
                                      name=f"pet{sb}{h}")
                        state[("pet", sb, h)] = pet
                    else:
                        pet = state[("pet", sb, h)]
                    for sc in scs:
                        t_end = s0 + sc * P + P
                        nblk = t_end // P + 1
                        pes = state[("pes", sb, h, sc)]
                        nc.sync.dma_start_transpose(
                            pet[:, :nblk, sc * P:(sc + 1) * P],
                            pes[:, :nblk * P])
                return go

            def b_av(sb, h, tag="acc"):
                def go():
                    s0 = sb * 512
                    pet = state[("pet", sb, h)]
                    recbc = wk.tile([P, 512], f16, tag="recbc", bufs=2,
                                    name=f"recbc{sb}{h}")
                    for sc in range(4):
                        nblk = (s0 + sc * P + P) // P + 1
                        nc.gpsimd.partition_broadcast(
                            recbc[:, sc * P:(sc + 1) * P],
                            pet[0:1, nblk - 1, sc * P:(sc + 1) * P])
                    ntile = 4 * sb + 4
                    po = psp.tile([P, 512], f32, tag=tag, bufs=2,
                                  name=f"po{sb}{h}")
                    for ti in range(ntile):
                        jj = max(0, ti - 4 * sb)
                        nc.tensor.matmul(po[:, jj * P:512],
                                         V[ti][:, h * P:(h + 1) * P],
                                         pet[:, ti, jj * P:512],
                                         start=(ti == 0), stop=(ti == ntile - 1))
                    oth = wk.tile([P, 512], f16, tag=f"ot{h}", bufs=1,
                                  name=f"ot{sb}{h}")
                    state[("ot", sb)][h] = oth
                    nc.vector.tensor_mul(oth[:], po[:], recbc[:])
                return go

            def c_unit(sb, jb):
                def go():
                    j0 = jb * 512
                    ot = state[("ot", sb)]
                    outt = wk.tile([P, 4, 512], f16, tag="outt", bufs=1,
                                   name=f"outt{sb}{jb}")
                    for st in range(4):
                        ppt = psp.tile([P, 512], f32, tag="acc", bufs=2,
                                       name=f"pp{sb}{jb}{st}")
                        for m in range(HL):
                            nc.tensor.matmul(ppt[:],
                                             ot[m][:, st * P:(st + 1) * P],
                                             woc[:, m, j0:j0 + 512],
                                             start=(m == 0), stop=(m == HL - 1))
                        if st % 2 == 0:
                            nc.scalar.activation(outt[:, st, :], ppt[:], Copy)
                        else:
                            nc.vector.tensor_copy(outt[:, st, :], ppt[:])
                    nc.sync.dma_start(out_r[sb, :, :, j0:j0 + 512], outt[:])
                return go

            def units_for(sb, tail=False):
                def pair(*fs):
                    def go():
                        for f in fs:
                            f()
                    return go
                return [
                    b_scores(sb, 0, (0, 1)),
                    b_scores(sb, 0, (2, 3)),
                    pair(b_scores(sb, 1, (0, 1)), b_transpose(sb, 0, (0, 1))),
                    pair(b_scores(sb, 1, (2, 3)), b_transpose(sb, 0, (2, 3))),
                    pair(b_scores(sb, 2, (0, 1)), b_transpose(sb, 1, (0, 1))),
                    pair(b_scores(sb, 2, (2, 3)), b_transpose(sb, 1, (2, 3))),
                    b_av(sb, 0),
                    pair(b_scores(sb, 3, (0, 1)), b_transpose(sb, 2, (0, 1))),
                    pair(b_scores(sb, 3, (2, 3)), b_transpose(sb, 2, (2, 3))),
                    b_av(sb, 1, "pa" if tail else "acc"),
                    b_transpose(sb, 3),
                    b_av(sb, 2),
                    b_av(sb, 3, "pa" if tail else "acc"),
                    c_unit(sb, 0), c_unit(sb, 1), c_unit(sb, 2), c_unit(sb, 3),
                ]

            # ---------------- schedule ----------------
            for sb in range(4):
                state[("q", sb)] = [wk.tile([P, 512], f16, tag=f"q{m}", bufs=2,
                                            name=f"q{m}_{sb}")
                                    for m in range(HL)]
                state[("ot", sb)] = [None] * HL

                kinds = ([("v", i, False) for i in range(4)] +
                         [("q", m, False) for m in range(4)] +
                         [("k", m, m == 3) for m in range(4)])
                groups = [a_group(sb, kd, ix, hf, lg)
                          for kd, ix, lg in kinds for hf in (0, 1)]

                if sb < 3:
                    units = units_for(sb - 1) if sb > 0 else []
                    k = 0
                    for gj, g in enumerate(groups):
                        if gj == 20:
                            load_xtc(sb + 1, 0)
                        g()
                        if gj == 22:
                            # after the last group's first half (reversed
                            # c-order): ch=1 readers done, safe to overwrite
                            load_xtc(sb + 1, 1)
                        while k < len(units) and (gj + 1) * len(units) // 24 > k:
                            units[k]()
                            k += 1
                else:
                    u2 = units_for(2)
                    u3 = units_for(3, tail=True)
                    g = groups
                    seq = (
                        [g[0], u2[0], g[1], u2[1], g[2], u2[2], g[3], u2[3],
                         g[4], u2[4], g[5], u2[5], g[6], u2[6], g[7], u2[7],
                         g[8], u2[8], g[9], u2[9], g[10], u2[10],
                         g[11], u2[11], g[12], u2[12], g[13], u2[13],
                         g[14], u2[14], g[15], u2[15], g[16], u2[16],
                         g[17], u3[0], g[18], u3[1],
                         g[19], u3[2], g[20], u3[3],
                         g[21], u3[4], g[22], u3[5],
                         g[23], u3[6], u3[7], u3[8], u3[9], u3[10],
                         u3[11], u3[12], u3[13], u3[14], u3[15], u3[16]])
                    for f in seq:
                        f()

    nc.compile()
    return nc


def _get_program():
    if "nc" not in _CACHE:
        _CACHE["nc"] = _build()
    return _CACHE["nc"]


def make_in_maps(x, Wq, bq, Wk, bk, Wv, bv, Wo, bo):
    xtb = [np.ascontiguousarray(x[b].T).astype(np.float16) for b in range(B)]
    WqT = np.ascontiguousarray(Wq.T).astype(np.float16)
    WkT = np.ascontiguousarray(Wk.T).astype(np.float16)
    WvT = np.ascontiguousarray(Wv.T).astype(np.float16)
    WoT = np.ascontiguousarray(Wo.T).astype(np.float16)
    maskd = np.where(np.triu(np.ones((P, P), dtype=bool), k=1),
                     np.float32(-100.0), np.float32(0.0))
    in_maps = []
    for core in range(8):
        b, hg = divmod(core, 4)
        ms = slice(hg * ML, (hg + 1) * ML)
        in_maps.append({
            "xt": xtb[b],
            "wqt": np.ascontiguousarray(WqT[:, ms]),
            "wkt": np.ascontiguousarray(WkT[:, ms]),
            "wvt": np.ascontiguousarray(WvT[:, ms]),
            "wot": np.ascontiguousarray(WoT[ms, :]),
            "bqs": np.ascontiguousarray(bq[ms] * SCALE).astype(np.float32),
            "bk": np.ascontiguousarray(bk[ms]).astype(np.float32),
            "maskd": maskd,
        })
    return in_maps


def run(inputs, trace=False):
    from concourse.bass_utils import run_bass_kernel_spmd

    nc = _get_program()
    in_maps = make_in_maps(
        inputs["x"], inputs["Wq"], inputs["bq"], inputs["Wk"], inputs["bk"],
        inputs["Wv"], inputs["bv"], inputs["Wo"], inputs["bo"])
    res = run_bass_kernel_spmd(nc, in_maps, core_ids=list(range(8)), trace=trace)
    partials = [np.asarray(res.results[c]["out"]).astype(np.float32)
                for c in range(8)]
    bo64 = (np.asarray(inputs["bo"], dtype=np.float64) +
            np.asarray(inputs["Wo"], dtype=np.float64)
            @ np.asarray(inputs["bv"], dtype=np.float64))
    full = np.empty((B, S, C), dtype=np.float32)
    for b in range(B):
        acc = np.sum(np.stack(partials[4 * b:4 * b + 4], 0), 0,
                     dtype=np.float64) + bo64
        full[b] = acc.astype(np.float32)
    return full, res


def kernel(**inputs):
    full, _ = run(inputs, trace=False)
    return full



# revision 43
# speedup vs baseline: 1.2218x; 1.2218x over previous
"""Multi-head causal attention (B=2, S=2048, C=2048, H=16) on 8 NeuronCores.

Sharding: 2-way data parallel over batch x 4-way tensor parallel over heads.
Core i handles batch b = i // 4 and heads [4*(i%4), 4*(i%4)+4).

Design (s-major scores, no denominator matmul):
  phase A: Q/K (d on partitions) and V (s on partitions) projections in f16,
           weights SBUF-cached via bulk sync-queue DMAs, c-inner PSUM
           accumulation (N=512 streams), emitted in 8-matmul halves for
           fine-grained interleaving. Q/K evacuate on ACT (Identity + bias
           AP, SCALE folded into Q), V on DVE. The V bias is folded into
           the host-side reduction (softmax rows sum to 1, so bv only
           contributes Wo @ bv to the output).
  phase B: scores s-major (stationary Q chunk [d,128], moving K) into
           [128,1024] two-bank PSUM pairs; causal mask via DVE add of a
           -100 tile on the diagonal 128-block; exp on ACT with accum_out
           (softmax denominator for free, one op per 1024 cols); the
           denominator reciprocal is appended as an extra f16 column and
           rides the xbar DMA transpose ([s,t] -> [t,s] blocks) that feeds
           the AV matmul (stationary V, N=512 streams); the reciprocal row
           is partition-broadcast (gpsimd) and multiplied in during PSUM
           evacuation (DVE).
  phase C: output projection partials; evacuation alternates ACT/DVE;
           merged f16 stores (one DMA per 512x512 block); bo + Wo @ bv are
           added on the host during the cross-core partial reduction.
  Schedule: B/C units of block sb-1 interleave 1:1 into the 24 phase-A
           half-groups of block sb; unit order keeps transpose(h+2) after
           AV(h) so the 2-deep pet rotation never serializes; sb=3 units
           interleave with phase A of sb=3 itself (hand-ordered tail);
           sb=2's c_units are deferred into the deep tail as the only
           independent PE filler for sb=3's exp->transpose->AV chain.
  Queues: sync = bulk loads + transposes + stores (transposes emitted one
           unit late so they reach the queue head with satisfied waits);
           gpsimd = small loads + broadcasts; ACT/DVE split the evacuation
           work to keep either in-order queue from convoying.
  Transposes MUST write contiguous destinations: pet is sc-major
           [P, 4, 17, P] because a non-contiguous xbar-transpose dst
           (mid-dim stride) breaks the >=4KB M2S concat; the resulting
           descriptor storm slowed every tensor-engine op by 1.2x
           (259ns vs 216ns per 512-wide matmul) for the whole kernel.
           The AV matmul absorbs the layout via a strided moving AP
           ([4,128] @ stride 2176), which costs nothing.
"""

import numpy as np

B, S, C, H = 2, 2048, 2048, 16
D = C // H            # 128 per-head dim
HL = 4                # heads per core
ML = HL * D           # 512 local channels
P = 128
NCT = C // P          # 16 contraction tiles
NT = S // P           # 16 key tiles
SCALE = 1.0 / float(np.sqrt(D))

_CACHE = {}


def _build():
    import concourse.bacc as bacc
    import concourse.mybir as mybir
    import concourse.tile as tile

    f32 = mybir.dt.float32
    f16 = mybir.dt.float16
    Exp = mybir.ActivationFunctionType.Exp
    Ident = mybir.ActivationFunctionType.Identity
    Copy = mybir.ActivationFunctionType.Copy
    add = mybir.AluOpType.add
    X = mybir.AxisListType.X

    nc = bacc.Bacc("TRN2", target_bir_lowering=False, debug=False, num_devices=8)

    xt = nc.dram_tensor("xt", [C, S], f16, kind="ExternalInput")       # x[b].T
    wqt = nc.dram_tensor("wqt", [C, ML], f16, kind="ExternalInput")    # Wq.T cols
    wkt = nc.dram_tensor("wkt", [C, ML], f16, kind="ExternalInput")
    wvt = nc.dram_tensor("wvt", [C, ML], f16, kind="ExternalInput")
    wot = nc.dram_tensor("wot", [ML, C], f16, kind="ExternalInput")    # Wo.T rows
    bqs = nc.dram_tensor("bqs", [ML], f32, kind="ExternalInput")       # bq * SCALE
    bk = nc.dram_tensor("bk", [ML], f32, kind="ExternalInput")
    maskd = nc.dram_tensor("maskd", [P, P], f32, kind="ExternalInput") # 0 / -100
    out = nc.dram_tensor("out", [S, C], f16, kind="ExternalOutput")

    xt_r = xt[:, :].rearrange("(c p) s -> p c s", p=P)    # [128, 16, 2048]
    wq_r = wqt[:, :].rearrange("(c p) m -> p c m", p=P)   # [128, 16, 512]
    wk_r = wkt[:, :].rearrange("(c p) m -> p c m", p=P)
    wv_r = wvt[:, :].rearrange("(c p) m -> p c m", p=P)
    wo_r = wot[:, :].rearrange("(m p) j -> p m j", p=P)   # [128, 4, 2048]
    out_r = out[:, :].rearrange("(g st p) j -> g p st j", p=P, st=4)  # [4, 128, 4, 2048]

    with tile.TileContext(nc) as tc:
        with tc.tile_pool(name="persist", bufs=1) as pp_, \
             tc.tile_pool(name="work", bufs=1) as wk, \
             tc.tile_pool(name="psp", bufs=1, space="PSUM") as psp:

            wvc = pp_.tile([P, NCT, ML], f16, tag="wvc", name="wvc")
            wqc = pp_.tile([P, NCT, ML], f16, tag="wqc", name="wqc")
            wkc = pp_.tile([P, NCT, ML], f16, tag="wkc", name="wkc")
            woc = pp_.tile([P, HL, C], f16, tag="woc", name="woc")
            K = [pp_.tile([P, S], f16, tag=f"k{m}", name=f"k{m}") for m in range(HL)]
            V = [pp_.tile([P, ML], f16, tag=f"v{t}", name=f"v{t}") for t in range(NT)]
            maskT = pp_.tile([P, P], f32, tag="maskT", name="maskT")
            bqs_t = [pp_.tile([P, 1], f32, tag=f"bq{m}", name=f"bq{m}") for m in range(HL)]
            bk_t = [pp_.tile([P, 1], f32, tag=f"bk{m}", name=f"bk{m}") for m in range(HL)]

            # small loads on gpsimd; bulk loads on the fast sync HWDGE path
            nc.gpsimd.dma_start(maskT[:], maskd[:, :])
            for m in range(HL):
                nc.gpsimd.dma_start(bqs_t[m][:], bqs[m * P:(m + 1) * P, None])
                nc.gpsimd.dma_start(bk_t[m][:], bk[m * P:(m + 1) * P, None])

            # xt cache: contraction-split halves (c 0..7 / 8..15) per s-block
            xtc_tiles = {}

            def load_xtc(sb, ch, quarters=False):
                tag = "xtcA" if ch == 0 else "xtcB"
                bufs = 2 if ch == 0 else 1
                t = wk.tile([P, 8, 512], f16, tag=tag, bufs=bufs,
                            name=f"xtc{sb}{ch}")
                xtc_tiles[(sb, ch)] = t
                s0 = sb * 512
                if quarters:
                    for q in range(2):
                        nc.sync.dma_start(
                            t[:, q * 4:(q + 1) * 4, :],
                            xt_r[:, ch * 8 + q * 4:ch * 8 + (q + 1) * 4,
                                 s0:s0 + 512])
                else:
                    nc.sync.dma_start(t[:], xt_r[:, ch * 8:(ch + 1) * 8,
                                                  s0:s0 + 512])
                return t

            # startup: wv/xt loads interleaved in first-use order.  The
            # first chunks are 128KB so the first matmul's inputs clear the
            # ~2.6us DMA completion latency as early as possible.
            t0 = wk.tile([P, 8, 512], f16, tag="xtcA", bufs=2, name="xtc00")
            xtc_tiles[(0, 0)] = t0
            for c in range(2):
                nc.sync.dma_start(wvc[:, c:c + 1, :], wv_r[:, c:c + 1, :])
                nc.sync.dma_start(t0[:, c:c + 1, :], xt_r[:, c:c + 1, 0:512])
            nc.sync.dma_start(wvc[:, 2:4, :], wv_r[:, 2:4, :])
            nc.sync.dma_start(t0[:, 2:4, :], xt_r[:, 2:4, 0:512])
            nc.sync.dma_start(wvc[:, 4:8, :], wv_r[:, 4:8, :])
            nc.sync.dma_start(t0[:, 4:8, :], xt_r[:, 4:8, 0:512])
            nc.sync.dma_start(wvc[:, 8:12, :], wv_r[:, 8:12, :])
            load_xtc(0, 1)
            nc.sync.dma_start(wvc[:, 12:16, :], wv_r[:, 12:16, :])
            # NOTE: keep ALL bulk loads on the single sync ring — FIFO
            # order there IS the priority mechanism.  Splitting loads
            # onto the Activation HWDGE ring makes the SDMA engines
            # round-robin between the two queues at packet granularity,
            # stealing bandwidth from the first-use-critical wv/xt loads
            # (measured +17us).
            nc.sync.dma_start(wqc[:, 0:8, :], wq_r[:, 0:8, :])
            nc.sync.dma_start(wqc[:, 8:16, :], wq_r[:, 8:16, :])
            nc.sync.dma_start(wkc[:, 0:8, :], wk_r[:, 0:8, :])
            nc.sync.dma_start(wkc[:, 8:16, :], wk_r[:, 8:16, :])
            nc.sync.dma_start(woc[:], wo_r)

            state = {}

            # ---------------- phase A groups (emitted in 8-MM halves) ----
            def a_group(sb, kind, idx, half, last_group=False):
                def go():
                    xa = xtc_tiles[(sb, 0)]
                    xb = xtc_tiles[(sb, 1)]
                    order = list(range(16))
                    if last_group:
                        order = list(range(8, 16)) + list(range(8))
                    if half == 0:
                        acc = psp.tile([P, 512], f32, tag="pa", bufs=2,
                                       name=f"pa_{sb}{kind}{idx}")
                        state[("pa", sb, kind, idx)] = acc
                    else:
                        acc = state[("pa", sb, kind, idx)]
                    for n in range(half * 8, half * 8 + 8):
                        c = order[n]
                        ch, c8 = divmod(c, 8)
                        xtile = xa if ch == 0 else xb
                        if kind == "v":
                            nc.tensor.matmul(acc[:],
                                             xtile[:, c8, idx * P:(idx + 1) * P],
                                             wvc[:, c, :],
                                             start=(n == 0), stop=(n == 15))
                        else:
                            wc = wqc if kind == "q" else wkc
                            nc.tensor.matmul(acc[:],
                                             wc[:, c, idx * P:(idx + 1) * P],
                                             xtile[:, c8, :],
                                             start=(n == 0), stop=(n == 15))
                    if half == 0:
                        return
                    if kind == "v":
                        nc.vector.tensor_copy(V[sb * 4 + idx][:], acc[:])
                    elif kind == "q":
                        qsb = state[("q", sb)]
                        nc.scalar.activation(qsb[idx][:], acc[:], Ident,
                                             bias=bqs_t[idx][:], scale=SCALE)
                    else:
                        s0 = sb * 512
                        nc.scalar.activation(K[idx][:, s0:s0 + 512], acc[:],
                                             Ident, bias=bk_t[idx][:])
                return go

            # ---------------- phase B/C units ----------------
            def b_scores(sb, h, scs=(0, 1, 2, 3)):
                """scores + exp + denominator for head h (no transpose)."""
                def go():
                    s0 = sb * 512
                    qsb = state[("q", sb)]
                    for sc in scs:
                        t_end = s0 + sc * P + P
                        ntt = (t_end + 511) // 512
                        pes = wk.tile([P, (14 + sc) * P], f16, tag=f"pes{sc}",
                                      bufs=2, name=f"pes{sb}{h}{sc}")
                        den4 = wk.tile([P, 2], f32, tag=f"den{sc}", bufs=2,
                                       name=f"den{sb}{h}{sc}")
                        rec1 = wk.tile([P, 1], f32, tag=f"rec{sc}", bufs=2,
                                       name=f"rec{sb}{h}{sc}")
                        ps = None
                        for tt in range(ntt):
                            w = min(512, t_end - tt * 512)
                            tp, off = divmod(tt, 2)
                            if off == 0:
                                ps = psp.tile([P, 1024], f32, tag="ps", bufs=2,
                                              name=f"ps{sb}{h}{sc}{tp}")
                            nc.tensor.matmul(ps[:, off * 512:off * 512 + w],
                                             qsb[h][:, sc * P:(sc + 1) * P],
                                             K[h][:, tt * 512:tt * 512 + w],
                                             start=True, stop=True)
                            if tt == ntt - 1:
                                w2 = off * 512 + w
                                nc.vector.tensor_add(ps[:, w2 - P:w2],
                                                     ps[:, w2 - P:w2], maskT[:])
                            if off == 1 or tt == ntt - 1:
                                w2 = off * 512 + w
                                nc.scalar.activation(
                                    pes[:, tp * 1024:tp * 1024 + w2],
                                    ps[:, :w2], Exp,
                                    accum_out=den4[:, tp:tp + 1])
                        ntp = (ntt + 1) // 2
                        if ntp > 1:
                            dsum = wk.tile([P, 1], f32, tag=f"dsum{sc}", bufs=2,
                                           name=f"dsum{sb}{h}{sc}")
                            nc.vector.tensor_reduce(dsum[:], den4[:, :ntp], X, add)
                            nc.vector.reciprocal(rec1[:], dsum[:])
                        else:
                            nc.vector.reciprocal(rec1[:], den4[:, 0:1])
                        nc.vector.tensor_copy(pes[:, t_end:t_end + 1], rec1[:])
                        state[("pes", sb, h, sc)] = pes
                return go

            def b_transpose(sb, h, scs=(0, 1, 2, 3)):
                """xbar transposes for head h (emitted one unit late).

                pet is sc-major [P, 4, 17, P] so each transpose writes a
                CONTIGUOUS slab: a non-contiguous xbar-transpose dst breaks
                the >=4KB M2S concat and the resulting descriptor storm
                slows every tensor-engine op by ~1.2x for the whole kernel.
                """
                def go():
                    s0 = sb * 512
                    if 0 in scs:
                        pet = wk.tile([P, 4, 17, P], f16, tag="pet", bufs=2,
                                      name=f"pet{sb}{h}")
                        state[("pet", sb, h)] = pet
                    else:
                        pet = state[("pet", sb, h)]
                    for sc in scs:
                        t_end = s0 + sc * P + P
                        nblk = t_end // P + 1
                        pes = state[("pes", sb, h, sc)]
                        nc.sync.dma_start_transpose(
                            pet[:, sc, :nblk, :],
                            pes[:, :nblk * P])
                return go

            def b_av(sb, h, tag="acc"):
                def go():
                    s0 = sb * 512
                    pet = state[("pet", sb, h)]
                    recbc = wk.tile([P, 512], f16, tag="recbc", bufs=1,
                                    name=f"recbc{sb}{h}")
                    for sc in range(4):
                        nblk = (s0 + sc * P + P) // P + 1
                        nc.gpsimd.partition_broadcast(
                            recbc[:, sc * P:(sc + 1) * P],
                            pet[0:1, sc, nblk - 1, :])
                    ntile = 4 * sb + 4
                    po = psp.tile([P, 512], f32, tag=tag, bufs=2,
                                  name=f"po{sb}{h}")
                    for ti in range(ntile):
                        jj = max(0, ti - 4 * sb)
                        nc.tensor.matmul(po[:, jj * P:512],
                                         V[ti][:, h * P:(h + 1) * P],
                                         pet[:, jj:, ti, :],
                                         start=(ti == 0), stop=(ti == ntile - 1))
                    oth = wk.tile([P, 512], f16, tag=f"ot{h}", bufs=2,
                                  name=f"ot{sb}{h}")
                    state[("ot", sb)][h] = oth
                    nc.vector.tensor_mul(oth[:], po[:], recbc[:])
                return go

            def c_unit(sb, jb, store=True, evac="alt", wide=False):
                # evac="dve": all evacuations on DVE (for tail fillers that
                # must not queue behind the sb=3 exp chain on ACT).
                # wide=True: ppt rotates across BOTH the acc and pa tags
                # (4 banks) -- only legal in the tail where phase A is done.
                def go():
                    j0 = jb * 512
                    ot = state[("ot", sb)]
                    for half in range(2):
                        outt = wk.tile([P, 2, 512], f16, tag="outt", bufs=2,
                                       name=f"outt{sb}{jb}{half}")
                        state[("outt", sb, jb, half)] = outt
                        for si in range(2):
                            st = half * 2 + si
                            tag = "pa" if (wide and st % 2 == 1) else "acc"
                            ppt = psp.tile([P, 512], f32, tag=tag, bufs=2,
                                           name=f"pp{sb}{jb}{st}")
                            for m in range(HL):
                                nc.tensor.matmul(ppt[:],
                                                 ot[m][:, st * P:(st + 1) * P],
                                                 woc[:, m, j0:j0 + 512],
                                                 start=(m == 0),
                                                 stop=(m == HL - 1))
                            if evac == "dve" or st % 2 == 1:
                                nc.vector.tensor_copy(outt[:, si, :], ppt[:])
                            else:
                                nc.scalar.activation(outt[:, si, :], ppt[:], Copy)
                        if store:
                            nc.sync.dma_start(
                                out_r[sb, :, 2 * half:2 * half + 2, j0:j0 + 512],
                                outt[:])
                return go

            def c_store(sb, jb, halves=(0, 1)):
                def go():
                    j0 = jb * 512
                    for half in halves:
                        outt = state[("outt", sb, jb, half)]
                        nc.sync.dma_start(
                            out_r[sb, :, 2 * half:2 * half + 2, j0:j0 + 512],
                            outt[:])
                return go



            def units_for(sb, tail=False):
                def pair(*fs):
                    def go():
                        for f in fs:
                            f()
                    return go
                return [
                    b_scores(sb, 0, (0, 1)),
                    b_scores(sb, 0, (2, 3)),
                    pair(b_scores(sb, 1, (0, 1)), b_transpose(sb, 0, (0, 1))),
                    pair(b_scores(sb, 1, (2, 3)), b_transpose(sb, 0, (2, 3))),
                    pair(b_scores(sb, 2, (0, 1)), b_transpose(sb, 1, (0, 1))),
                    pair(b_scores(sb, 2, (2, 3)), b_transpose(sb, 1, (2, 3))),
                    b_av(sb, 0),
                    pair(b_scores(sb, 3, (0, 1)), b_transpose(sb, 2, (0, 1))),
                    pair(b_scores(sb, 3, (2, 3)), b_transpose(sb, 2, (2, 3))),
                    b_av(sb, 1, "pa" if tail else "acc"),
                    b_transpose(sb, 3),
                    b_av(sb, 2),
                    b_av(sb, 3, "pa" if tail else "acc"),
                    c_unit(sb, 0), c_unit(sb, 1),
                    c_unit(sb, 2), c_unit(sb, 3),
                ]

            # ---------------- schedule ----------------
            for sb in range(4):
                state[("q", sb)] = [wk.tile([P, 512], f16, tag=f"q{m}", bufs=2,
                                            name=f"q{m}_{sb}")
                                    for m in range(HL)]
                state[("ot", sb)] = [None] * HL

                kinds = ([("v", i, False) for i in range(4)] +
                         [("q", m, False) for m in range(4)] +
                         [("k", m, m == 3) for m in range(4)])
                groups = [a_group(sb, kd, ix, hf, lg)
                          for kd, ix, lg in kinds for hf in (0, 1)]

                if sb < 3:
                    units = units_for(sb - 1) if sb > 0 else []
                    k = 0
                    for gj, g in enumerate(groups):
                        if gj == 20:
                            load_xtc(sb + 1, 0)
                        g()
                        if gj == 22:
                            # after the last group's first half (reversed
                            # c-order): ch=1 readers done, safe to overwrite
                            load_xtc(sb + 1, 1)
                        while k < len(units) and (gj + 1) * len(units) // 24 > k:
                            units[k]()
                            k += 1
                else:
                    # sb=2's c_units are DEFERRED into the deep tail: they are
                    # the only fully-independent PE work available to fill the
                    # bubbles in sb=3's exp->transpose->broadcast->AV chain
                    # (interleaving them with sb=3's phase A, as before, only
                    # competed PE-work with PE-work).  Their stores are placed
                    # after the critical tail transposes on the sync ring.
                    u2 = units_for(2)[:13]
                    c2 = [c_unit(2, jb, store=False) for jb in range(4)]
                    s2 = [c_store(2, jb) for jb in range(4)]
                    u3 = units_for(3, tail=True)
                    g = groups
                    seq = (
                        [g[0], u2[0], g[1], u2[1], g[2], u2[2], g[3], u2[3],
                         g[4], u2[4], g[5], u2[5], g[6], u2[6], g[7], u2[7],
                         g[8], u2[8], g[9], u2[9], g[10], u2[10],
                         g[11], u2[11], g[12], u2[12], g[13],
                         g[14], g[15], g[16],
                         g[17], u3[0], g[18], u3[1],
                         g[19], u3[2], g[20], u3[3],
                         g[21], u3[4], g[22], u3[5],
                         g[23], u3[6], u3[7], u3[8], u3[9],
                         c2[0], u3[10], c2[1], u3[11],
                         s2[0], c2[2], u3[12], s2[1], c2[3],
                         u3[13], s2[2], u3[14], s2[3], u3[15], u3[16]])
                    for f in seq:
                        f()

    nc.compile()
    return nc


def _get_program():
    if "nc" not in _CACHE:
        _CACHE["nc"] = _build()
    return _CACHE["nc"]


def make_in_maps(x, Wq, bq, Wk, bk, Wv, bv, Wo, bo):
    xtb = [np.ascontiguousarray(x[b].T).astype(np.float16) for b in range(B)]
    WqT = np.ascontiguousarray(Wq.T).astype(np.float16)
    WkT = np.ascontiguousarray(Wk.T).astype(np.float16)
    WvT = np.ascontiguousarray(Wv.T).astype(np.float16)
    WoT = np.ascontiguousarray(Wo.T).astype(np.float16)
    maskd = np.where(np.triu(np.ones((P, P), dtype=bool), k=1),
                     np.float32(-100.0), np.float32(0.0))
    in_maps = []
    for core in range(8):
        b, hg = divmod(core, 4)
        ms = slice(hg * ML, (hg + 1) * ML)
        in_maps.append({
            "xt": xtb[b],
            "wqt": np.ascontiguousarray(WqT[:, ms]),
            "wkt": np.ascontiguousarray(WkT[:, ms]),
            "wvt": np.ascontiguousarray(WvT[:, ms]),
            "wot": np.ascontiguousarray(WoT[ms, :]),
            "bqs": np.ascontiguousarray(bq[ms] * SCALE).astype(np.float32),
            "bk": np.ascontiguousarray(bk[ms]).astype(np.float32),
            "maskd": maskd,
        })
    return in_maps


def run(inputs, trace=False):
    from concourse.bass_utils import run_bass_kernel_spmd

    nc = _get_program()
    in_maps = make_in_maps(
        inputs["x"], inputs["Wq"], inputs["bq"], inputs["Wk"], inputs["bk"],
        inputs["Wv"], inputs["bv"], inputs["Wo"], inputs["bo"])
    res = run_bass_kernel_spmd(nc, in_maps, core_ids=list(range(8)), trace=trace)
    partials = [np.asarray(res.results[c]["out"]).astype(np.float32)
                for c in range(8)]
    bo64 = (np.asarray(inputs["bo"], dtype=np.float64) +
            np.asarray(inputs["Wo"], dtype=np.float64)
            @ np.asarray(inputs["bv"], dtype=np.float64))
    full = np.empty((B, S, C), dtype=np.float32)
    for b in range(B):
        acc = np.sum(np.stack(partials[4 * b:4 * b + 4], 0), 0,
                     dtype=np.float64) + bo64
        full[b] = acc.astype(np.float32)
    return full, res


def kernel(**inputs):
    full, _ = run(inputs, trace=False)
    return full

